# revision 24
# baseline (speedup 1.0000x reference)
"""Trainium2 Bass kernel for multi-head attention (B=4, N=2048, C=1024, H=16).

Sharding (8 cores, no collectives): core c handles batch b = c//2 and
sequence-half h2 = c%2 (q rows [h2*1024, h2*1024+1024)). Each core computes
k/v for the full sequence of its batch (duplicated within the pair), its
q-half, attention for all 16 heads, and the output projection for its rows.
Host concatenates the 8 row-blocks. Host rotates x columns so the local
q-half is always cols [0, 1024) (k-order permutation is softmax-invariant).

Engine layout per core:
  - all matmul operands fp16 (full PE rate + fast weight load); PSUM fp32.
  - x^T resident in SBUF [1024, 2048] fp16; QKV reads it directly.
  - scores^T [128k, 1024q] fp32 PSUM chunks -> ACT exp -> PT fp16 SBUF.
  - attn@V via PE with v augmented by a ones column (denominator for free);
    ctx^T accumulated in PSUM [65, 1024].
  - softmax normalize: DVE copy denom -> gpsimd partition_broadcast ->
    DVE reciprocal_approx_fast -> DVE multiply -> ctx^T fp16 resident.
  - no max-subtraction: scores are ~N(0, 0.17), exp cannot overflow.
"""
import sys

sys.path.insert(0, "/opt/trn_rl_repo")

import numpy as np

B, N, C = 4, 2048, 1024
H = 16
D = C // H
SCALE = np.float32(1.0) / np.sqrt(D).astype(np.float32)
NCORES = 8
NLOC = N // 2            # q rows per core
NKC = N // 128           # 16 k-chunks
NPAIR = H // 2           # 8 head pairs
VW = 65                  # v columns + ones
_cache = {}


def _build():
    import concourse.bacc as bacc
    import concourse.tile as tile
    import concourse.mybir as mybir

    f32 = mybir.dt.float32
    f16 = mybir.dt.float16

    nc = bacc.Bacc("TRN2", target_bir_lowering=False, debug=False,
                   num_devices=NCORES)

    xT_d = nc.dram_tensor("xT", [128, 8 * N], f16, kind="ExternalInput").ap()
    wq_d = nc.dram_tensor("wq", [128, 8 * C], f16, kind="ExternalInput").ap()
    wk_d = nc.dram_tensor("wk", [128, 8 * C], f16, kind="ExternalInput").ap()
    wv_d = nc.dram_tensor("wv", [128, 8 * C], f16, kind="ExternalInput").ap()
    wo_d = nc.dram_tensor("wo", [128, 8 * C], f16, kind="ExternalInput").ap()
    bo_d = nc.dram_tensor("bo_b", [128, C], f32, kind="ExternalInput").ap()
    out_d = nc.dram_tensor("out", [NLOC, C], f32, kind="ExternalOutput").ap()

    with tile.TileContext(nc) as tc:
        with tc.tile_pool(name="mm_ps", bufs=2, space="PSUM") as mm_ps, \
             tc.tile_pool(name="sc_ps", bufs=2, space="PSUM") as sc_ps, \
             tc.tile_pool(name="ctx_ps", bufs=1, space="PSUM") as ctx_ps, \
             tc.tile_pool(name="big", bufs=1) as big, \
             tc.tile_pool(name="w_pool", bufs=2) as w_pool, \
             tc.tile_pool(name="kq_pool", bufs=2) as kq_pool, \
             tc.tile_pool(name="pt_pool", bufs=34) as pt_pool, \
             tc.tile_pool(name="nrm_pool", bufs=1) as nrm_pool, \
             tc.tile_pool(name="out_pool", bufs=3) as out_pool:
            # resident tensors
            XT = big.tile([128, 8 * N], f16, name="XT")      # [cc, 128c, n]
            VA = big.tile([128, H * NKC * VW], f16, name="VA")
            nc.vector.memset(
                VA[:].bitcast(f16).rearrange("p (hk w) -> p hk w",
                                             w=VW)[:, :, 64], 1.0)
            ctxT = big.tile([128, NPAIR * NLOC], f16, name="ctxT")

            # ---- V for all heads (dh-inner: stationary XT chunk reused) ----
            wv_ctx = tc.tile_pool(name="wv_pool", bufs=1)
            wv_pool = wv_ctx.__enter__()
            WV = wv_pool.tile([128, 8 * C], f16, name="WV")
            # small head slices first so the first v matmuls can start early
            for cc in range(8):
                eng = (nc.sync, nc.gpsimd, nc.scalar)[cc % 3]
                eng.dma_start(out=XT[:, cc * N:cc * N + 512],
                              in_=xT_d[:, cc * N:cc * N + 512])
                nc.sync.dma_start(out=WV[:, cc * C:cc * C + 512],
                                  in_=wv_d[:, cc * C:cc * C + 512])
            for cc in range(8):
                eng = (nc.sync, nc.gpsimd, nc.scalar)[cc % 3]
                eng.dma_start(
                    out=XT[:, cc * N + 512:(cc + 1) * N],
                    in_=xT_d[:, cc * N + 512:(cc + 1) * N])
                nc.sync.dma_start(out=WV[:, cc * C + 512:(cc + 1) * C],
                                  in_=wv_d[:, cc * C + 512:(cc + 1) * C])
            def emit_v():
                for nsub in range(NKC):
                    for dh in range(2):
                        ps = mm_ps.tile([128, 512], f32, name="ps")
                        for cc in range(8):
                            nc.tensor.matmul(
                                ps[:],
                                lhsT=XT[:, cc * N + nsub * 128:
                                        cc * N + (nsub + 1) * 128],
                                rhs=WV[:, cc * C + dh * 512:
                                       cc * C + dh * 512 + 512],
                                start=(cc == 0), stop=(cc == 7))
                        nc.vector.tensor_copy(
                            VA[:].rearrange("p (h c) -> p h c", h=H)
                               [:, dh * 8:(dh + 1) * 8,
                                nsub * VW:nsub * VW + 64],
                            ps[:].rearrange("p (h d) -> p h d", h=8))

            # ---- per head-pair: qT/kT production + attention ----
            def emit_qkv(hp):
                wk_t = w_pool.tile([128, 8 * 128], f16, name="wkp")
                nc.sync.dma_start(
                    out=wk_t[:], in_=wk_d[:, hp * 1024:(hp + 1) * 1024])
                wq_t = w_pool.tile([128, 8 * 128], f16, name="wqp")
                nc.sync.dma_start(
                    out=wq_t[:], in_=wq_d[:, hp * 1024:(hp + 1) * 1024])
                kT = kq_pool.tile([128, N], f16, name="kT")
                for nb in range(N // 512):
                    ps = mm_ps.tile([128, 512], f32, name="ps")
                    for cc in range(8):
                        nc.tensor.matmul(
                            ps[:], lhsT=wk_t[:, cc * 128:(cc + 1) * 128],
                            rhs=XT[:, cc * N + nb * 512:cc * N + (nb + 1) * 512],
                            start=(cc == 0), stop=(cc == 7))
                    nc.vector.tensor_copy(kT[:, nb * 512:(nb + 1) * 512], ps[:])
                qT = kq_pool.tile([128, NLOC], f16, name="qT")
                for nb in range(NLOC // 512):
                    ps = mm_ps.tile([128, 512], f32, name="ps")
                    for cc in range(8):
                        nc.tensor.matmul(
                            ps[:], lhsT=wq_t[:, cc * 128:(cc + 1) * 128],
                            rhs=XT[:, cc * N + nb * 512:cc * N + (nb + 1) * 512],
                            start=(cc == 0), stop=(cc == 7))
                    nc.vector.tensor_copy(qT[:, nb * 512:(nb + 1) * 512], ps[:])
                return kT, qT

            kq_next = emit_qkv(0)
            for hp in range(NPAIR):
                kT, qT = kq_next

                # scores+exp: both heads of the pair share one PSUM tile per
                # (kc, j): h0 -> cols 0-511 (rows 0-63), h64 -> cols 512-1023
                # (rows 64-127). The two matmuls hit disjoint PE row groups and
                # stream concurrently (~2x scores throughput); one exp covers
                # both.
                pts = {}
                for kc in range(NKC):
                    for j in range(NLOC // 512):
                        sp = sc_ps.tile([128, NLOC], f32, name="sc")
                        for hh in range(2):
                            r0 = hh * 64
                            nc.tensor.matmul(
                                sp[:, hh * 512:(hh + 1) * 512],
                                lhsT=kT[r0:r0 + 64, kc * 128:(kc + 1) * 128],
                                rhs=qT[r0:r0 + 64, j * 512:(j + 1) * 512],
                                start=True, stop=True)
                        pt = pt_pool.tile([128, NLOC], f16, name="pt")
                        nc.scalar.activation(
                            pt[:], sp[:], mybir.ActivationFunctionType.Exp)
                        pts[(kc, j)] = pt
                if hp == 0:
                    # v-production emitted here: its matmuls fill the PE while
                    # ACT chews through pair 0's exps; attnV below needs VA.
                    emit_v()
                    wv_ctx.__exit__(None, None, None)
                if hp + 1 < NPAIR:
                    # next pair's QKV emitted BEFORE this pair's attnV: the
                    # scheduler slots it into this pair's exp-paced PE gaps so
                    # scores(hp+1) are ready the moment ACT drains pair hp.
                    kq_next = emit_qkv(hp + 1)

                def emit_attnv(ahp, akT, aqT, apts):
                    for hh in range(2):
                        h = ahp * 2 + hh
                        r0 = hh * 64
                        ctx_p = ctx_ps.tile([VW, NLOC], f32)
                        for kc in range(NKC):
                            for j in range(NLOC // 512):
                                nc.tensor.matmul(
                                        ctx_p[:, j * 512:(j + 1) * 512],
                                        lhsT=VA[:, (h * NKC + kc) * VW:
                                                (h * NKC + kc + 1) * VW],
                                        rhs=apts[(kc, j)][:, hh * 512:(hh + 1) * 512],
                                        start=(kc == 0), stop=(kc == NKC - 1))
                        den = nrm_pool.tile([1, NLOC], f32, name="den")
                        nc.vector.tensor_copy(den[:], ctx_p[64:65, :])
                        den_b = nrm_pool.tile([64, NLOC], f32, name="den_b")
                        nc.gpsimd.partition_broadcast(den_b[:], den[:])
                        rec = nrm_pool.tile([64, NLOC], f32, name="rec")
                        nc.vector.reciprocal_approx_fast(out=rec[:], in_=den_b[:])
                        nc.vector.tensor_tensor(
                            out=ctxT[r0:r0 + 64, ahp * NLOC:(ahp + 1) * NLOC],
                            in0=ctx_p[:64, :], in1=rec[:],
                            op=mybir.AluOpType.mult)


                if hp > 0:
                    emit_attnv(*prev_state)
                prev_state = (hp, kT, qT, pts)

            emit_attnv(*prev_state)

            # ---- projection ----
            wo_ctx = tc.tile_pool(name="wo_pool", bufs=1)
            wo_pool = wo_ctx.__enter__()
            WO = wo_pool.tile([128, 8 * C], f16, name="WO")
            nc.sync.dma_start(out=WO[:], in_=wo_d)
            BO = wo_pool.tile([128, C], f32, name="BO")
            nc.sync.dma_start(out=BO[:], in_=bo_d)
            for nt in range(NLOC // 128):
                for ch in range(2):
                    ps = mm_ps.tile([128, 512], f32, name="ps")
                    for cc in range(8):
                        nc.tensor.matmul(
                            ps[:],
                            lhsT=ctxT[:, cc * NLOC + nt * 128:
                                      cc * NLOC + nt * 128 + 128],
                            rhs=WO[:, cc * C + ch * 512:cc * C + ch * 512 + 512],
                            start=(cc == 0), stop=(cc == 7))
                    ot = out_pool.tile([128, 512], f32)
                    nc.vector.tensor_tensor(
                        out=ot[:], in0=ps[:], in1=BO[:, ch * 512:(ch + 1) * 512],
                        op=mybir.AluOpType.add)
                    nc.gpsimd.dma_start(
                        out=out_d[nt * 128:(nt + 1) * 128,
                                  ch * 512:(ch + 1) * 512],
                        in_=ot[:])
            wo_ctx.__exit__(None, None, None)

    nc.compile()
    return nc


def kernel(x, Wq, Wk, Wv, Wo, bo, _trace=False):
    from concourse.bass_utils import run_bass_kernel_spmd

    if "nc" not in _cache:
        _cache["nc"] = _build()
    nc = _cache["nc"]

    def _chunked(w):
        # [C, C] -> [128, 8*C]: row p holds w[cc*128+p, :] for cc = 0..7
        return np.ascontiguousarray(
            np.asarray(w, dtype=np.float32).astype(np.float16)
            .reshape(8, 128, C).transpose(1, 0, 2).reshape(128, 8 * C))

    def _pair_chunked(w):
        # [C, C] -> [128, (hp, cc, 128)]: per head-pair contiguous blocks
        a = (np.asarray(w, dtype=np.float32).astype(np.float16)
             .reshape(8, 128, 8, 128))           # [cc, p, hp, d]
        return np.ascontiguousarray(
            a.transpose(1, 2, 0, 3).reshape(128, 8 * C))

    x = np.asarray(x, dtype=np.float32)
    wq = _pair_chunked(np.asarray(Wq, dtype=np.float32) * SCALE)
    wk = _pair_chunked(Wk)
    wv = _chunked(Wv)
    wo = _chunked(Wo)
    bo_b = np.ascontiguousarray(
        np.broadcast_to(np.asarray(bo, dtype=np.float32), (128, C)))

    in_maps = []
    for c in range(NCORES):
        b, h2 = divmod(c, 2)
        xT = x[b].T.astype(np.float16)
        # rotate so the local q-half is cols [0, NLOC); chunk to [128, 8*N]
        xT_rot = np.roll(xT, -h2 * NLOC, axis=1)
        xT_c = np.ascontiguousarray(
            xT_rot.reshape(8, 128, N).transpose(1, 0, 2).reshape(128, 8 * N))
        in_maps.append({"xT": xT_c, "wq": wq, "wk": wk, "wv": wv,
                        "wo": wo, "bo_b": bo_b})

    res = run_bass_kernel_spmd(nc, in_maps, core_ids=list(range(NCORES)),
                               trace=_trace, trace_cores=[0] if _trace else None)
    out = np.empty((B, N, C), dtype=np.float32)
    for c in range(NCORES):
        b, h2 = divmod(c, 2)
        out[b, h2 * NLOC:(h2 + 1) * NLOC, :] = res.results[c]["out"]
    if _trace:
        _cache["last_trace"] = res
    return out


# revision 25
# speedup vs baseline: 1.1002x; 1.1002x over previous
"""Trainium2 Bass kernel for multi-head attention (B=4, N=2048, C=1024, H=16).

Sharding (8 cores, no collectives): core c handles batch b = c//2 and
sequence-half h2 = c%2 (q rows [h2*1024, h2*1024+1024)). Each core computes
k/v for the full sequence of its batch (duplicated within the pair), its
q-half, attention for all 16 heads, and the output projection for its rows.
Host concatenates the 8 row-blocks. Host rotates x columns so the local
q-half is always cols [0, 1024) (k-order permutation is softmax-invariant).

Engine layout per core:
  - all matmul operands fp16 (full PE rate + fast weight load); PSUM fp32.
  - x^T resident in SBUF [1024, 2048] fp16; QKV reads it directly.
  - scores^T [128k, 1024q] fp32 PSUM chunks -> ACT exp -> PT fp16 SBUF.
  - attn@V via PE with v augmented by a ones column (denominator for free);
    ctx^T accumulated in PSUM [65, 1024].
  - softmax normalize: DVE copy denom -> gpsimd partition_broadcast ->
    DVE reciprocal_approx_fast -> DVE multiply -> ctx^T fp16 resident.
  - no max-subtraction: scores are ~N(0, 0.17), exp cannot overflow.
"""
import sys

sys.path.insert(0, "/opt/trn_rl_repo")

import numpy as np

B, N, C = 4, 2048, 1024
H = 16
D = C // H
SCALE = np.float32(1.0) / np.sqrt(D).astype(np.float32)
NCORES = 8
NLOC = N // 2            # q rows per core
NKC = N // 128           # 16 k-chunks
NPAIR = H // 2           # 8 head pairs
VW = 65                  # v columns + ones
_cache = {}


def _build():
    import concourse.bacc as bacc
    import concourse.tile as tile
    import concourse.mybir as mybir

    f32 = mybir.dt.float32
    f16 = mybir.dt.float16

    nc = bacc.Bacc("TRN2", target_bir_lowering=False, debug=False,
                   num_devices=NCORES)

    xT_d = nc.dram_tensor("xT", [128, 8 * N], f16, kind="ExternalInput").ap()
    wq_d = nc.dram_tensor("wq", [128, 8 * C], f16, kind="ExternalInput").ap()
    wk_d = nc.dram_tensor("wk", [128, 8 * C], f16, kind="ExternalInput").ap()
    wv_d = nc.dram_tensor("wv", [128, 8 * C], f16, kind="ExternalInput").ap()
    wo_d = nc.dram_tensor("wo", [128, 8 * C], f16, kind="ExternalInput").ap()
    bo_d = nc.dram_tensor("bo_b", [128, C], f32, kind="ExternalInput").ap()
    out_d = nc.dram_tensor("out", [NLOC, C], f32, kind="ExternalOutput").ap()

    with tile.TileContext(nc) as tc:
        with tc.tile_pool(name="mm_ps", bufs=2, space="PSUM") as mm_ps, \
             tc.tile_pool(name="sc_ps", bufs=2, space="PSUM") as sc_ps, \
             tc.tile_pool(name="ctx_ps", bufs=1, space="PSUM") as ctx_ps, \
             tc.tile_pool(name="big", bufs=1) as big, \
             tc.tile_pool(name="w_pool", bufs=2) as w_pool, \
             tc.tile_pool(name="kq_pool", bufs=2) as kq_pool, \
             tc.tile_pool(name="pt_pool", bufs=34) as pt_pool, \
             tc.tile_pool(name="nrm_pool", bufs=1) as nrm_pool, \
             tc.tile_pool(name="out_pool", bufs=3) as out_pool:
            # resident tensors
            XT = big.tile([128, 8 * N], f16, name="XT")      # [cc, 128c, n]
            VA = big.tile([128, H * NKC * VW], f16, name="VA")
            nc.vector.memset(
                VA[:].bitcast(f16).rearrange("p (hk w) -> p hk w",
                                             w=VW)[:, :, 64], 1.0)
            ctxT = big.tile([128, NPAIR * NLOC], f16, name="ctxT")

            # ---- V for all heads (dh-inner: stationary XT chunk reused) ----
            wv_ctx = tc.tile_pool(name="wv_pool", bufs=1)
            wv_pool = wv_ctx.__enter__()
            WV = wv_pool.tile([128, 8 * C], f16, name="WV")
            # small head slices first so the first v matmuls can start early
            for cc in range(8):
                eng = (nc.sync, nc.gpsimd, nc.scalar)[cc % 3]
                eng.dma_start(out=XT[:, cc * N:cc * N + 512],
                              in_=xT_d[:, cc * N:cc * N + 512])
                nc.sync.dma_start(out=WV[:, cc * C:cc * C + 512],
                                  in_=wv_d[:, cc * C:cc * C + 512])
            for cc in range(8):
                eng = (nc.sync, nc.gpsimd, nc.scalar)[cc % 3]
                eng.dma_start(
                    out=XT[:, cc * N + 512:(cc + 1) * N],
                    in_=xT_d[:, cc * N + 512:(cc + 1) * N])
                nc.sync.dma_start(out=WV[:, cc * C + 512:(cc + 1) * C],
                                  in_=wv_d[:, cc * C + 512:(cc + 1) * C])
            for nsub in range(NKC):
                for dh in range(2):
                    ps = mm_ps.tile([128, 512], f32, name="ps")
                    for cc in range(8):
                        nc.tensor.matmul(
                            ps[:],
                            lhsT=XT[:, cc * N + nsub * 128:
                                    cc * N + (nsub + 1) * 128],
                            rhs=WV[:, cc * C + dh * 512:cc * C + dh * 512 + 512],
                            start=(cc == 0), stop=(cc == 7))
                    nc.vector.tensor_copy(
                        VA[:].rearrange("p (h c) -> p h c", h=H)
                           [:, dh * 8:(dh + 1) * 8, nsub * VW:nsub * VW + 64],
                        ps[:].rearrange("p (h d) -> p h d", h=8))

            wv_ctx.__exit__(None, None, None)

            # ---- per head-pair: qT/kT production + attention ----
            for hp in range(NPAIR):
                wk_t = w_pool.tile([128, 8 * 128], f16, name="wkp")
                nc.sync.dma_start(
                    out=wk_t[:], in_=wk_d[:, hp * 1024:(hp + 1) * 1024])
                wq_t = w_pool.tile([128, 8 * 128], f16, name="wqp")
                nc.sync.dma_start(
                    out=wq_t[:], in_=wq_d[:, hp * 1024:(hp + 1) * 1024])

                kT = kq_pool.tile([128, N], f16, name="kT")
                for nb in range(N // 512):
                    ps = mm_ps.tile([128, 512], f32, name="ps")
                    for cc in range(8):
                        nc.tensor.matmul(
                            ps[:], lhsT=wk_t[:, cc * 128:(cc + 1) * 128],
                            rhs=XT[:, cc * N + nb * 512:cc * N + (nb + 1) * 512],
                            start=(cc == 0), stop=(cc == 7))
                    nc.vector.tensor_copy(kT[:, nb * 512:(nb + 1) * 512], ps[:])
                qT = kq_pool.tile([128, NLOC], f16, name="qT")
                for nb in range(NLOC // 512):
                    ps = mm_ps.tile([128, 512], f32, name="ps")
                    for cc in range(8):
                        nc.tensor.matmul(
                            ps[:], lhsT=wq_t[:, cc * 128:(cc + 1) * 128],
                            rhs=XT[:, cc * N + nb * 512:cc * N + (nb + 1) * 512],
                            start=(cc == 0), stop=(cc == 7))
                    nc.vector.tensor_copy(qT[:, nb * 512:(nb + 1) * 512], ps[:])

                # scores+exp: both heads of the pair share one PSUM tile per
                # (kc, j): h0 -> cols 0-511 (rows 0-63), h64 -> cols 512-1023
                # (rows 64-127). The two matmuls hit disjoint PE row groups and
                # stream concurrently (~2x scores throughput); one exp covers
                # both.
                pts = {}
                for kc in range(NKC):
                    for j in range(NLOC // 512):
                        sp = sc_ps.tile([128, NLOC], f32, name="sc")
                        for hh in range(2):
                            r0 = hh * 64
                            nc.tensor.matmul(
                                sp[:, hh * 512:(hh + 1) * 512],
                                lhsT=kT[r0:r0 + 64, kc * 128:(kc + 1) * 128],
                                rhs=qT[r0:r0 + 64, j * 512:(j + 1) * 512],
                                start=True, stop=True)
                        pt = pt_pool.tile([128, NLOC], f16, name="pt")
                        nc.scalar.activation(
                            pt[:], sp[:], mybir.ActivationFunctionType.Exp)
                        pts[(kc, j)] = pt
                for hh in range(2):
                    h = hp * 2 + hh
                    r0 = hh * 64
                    ctx_p = ctx_ps.tile([VW, NLOC], f32)
                    for kc in range(NKC):
                        for j in range(NLOC // 512):
                            nc.tensor.matmul(
                                ctx_p[:, j * 512:(j + 1) * 512],
                                lhsT=VA[:, (h * NKC + kc) * VW:
                                        (h * NKC + kc + 1) * VW],
                                rhs=pts[(kc, j)][:, hh * 512:(hh + 1) * 512],
                                start=(kc == 0), stop=(kc == NKC - 1))
                    den = nrm_pool.tile([1, NLOC], f32, name="den")
                    nc.vector.tensor_copy(den[:], ctx_p[64:65, :])
                    den_b = nrm_pool.tile([64, NLOC], f32, name="den_b")
                    nc.gpsimd.partition_broadcast(den_b[:], den[:])
                    rec = nrm_pool.tile([64, NLOC], f32, name="rec")
                    nc.vector.reciprocal_approx_fast(out=rec[:], in_=den_b[:])
                    nc.vector.tensor_tensor(
                        out=ctxT[r0:r0 + 64, hp * NLOC:(hp + 1) * NLOC],
                        in0=ctx_p[:64, :], in1=rec[:],
                        op=mybir.AluOpType.mult)

            # ---- projection ----
            wo_ctx = tc.tile_pool(name="wo_pool", bufs=1)
            wo_pool = wo_ctx.__enter__()
            WO = wo_pool.tile([128, 8 * C], f16, name="WO")
            nc.sync.dma_start(out=WO[:], in_=wo_d)
            BO = wo_pool.tile([128, C], f32, name="BO")
            nc.sync.dma_start(out=BO[:], in_=bo_d)
            for nt in range(NLOC // 128):
                for ch in range(2):
                    ps = mm_ps.tile([128, 512], f32, name="ps")
                    for cc in range(8):
                        nc.tensor.matmul(
                            ps[:],
                            lhsT=ctxT[:, cc * NLOC + nt * 128:
                                      cc * NLOC + nt * 128 + 128],
                            rhs=WO[:, cc * C + ch * 512:cc * C + ch * 512 + 512],
                            start=(cc == 0), stop=(cc == 7))
                    ot = out_pool.tile([128, 512], f32)
                    nc.vector.tensor_tensor(
                        out=ot[:], in0=ps[:], in1=BO[:, ch * 512:(ch + 1) * 512],
                        op=mybir.AluOpType.add)
                    nc.gpsimd.dma_start(
                        out=out_d[nt * 128:(nt + 1) * 128,
                                  ch * 512:(ch + 1) * 512],
                        in_=ot[:])
            wo_ctx.__exit__(None, None, None)

    nc.compile()
    return nc


def kernel(x, Wq, Wk, Wv, Wo, bo, _trace=False):
    from concourse.bass_utils import run_bass_kernel_spmd

    if "nc" not in _cache:
        _cache["nc"] = _build()
    nc = _cache["nc"]

    def _chunked(w):
        # [C, C] -> [128, 8*C]: row p holds w[cc*128+p, :] for cc = 0..7
        return np.ascontiguousarray(
            np.asarray(w, dtype=np.float32).astype(np.float16)
            .reshape(8, 128, C).transpose(1, 0, 2).reshape(128, 8 * C))

    def _pair_chunked(w):
        # [C, C] -> [128, (hp, cc, 128)]: per head-pair contiguous blocks
        a = (np.asarray(w, dtype=np.float32).astype(np.float16)
             .reshape(8, 128, 8, 128))           # [cc, p, hp, d]
        return np.ascontiguousarray(
            a.transpose(1, 2, 0, 3).reshape(128, 8 * C))

    x = np.asarray(x, dtype=np.float32)
    wq = _pair_chunked(np.asarray(Wq, dtype=np.float32) * SCALE)
    wk = _pair_chunked(Wk)
    wv = _chunked(Wv)
    wo = _chunked(Wo)
    bo_b = np.ascontiguousarray(
        np.broadcast_to(np.asarray(bo, dtype=np.float32), (128, C)))

    in_maps = []
    for c in range(NCORES):
        b, h2 = divmod(c, 2)
        xT = x[b].T.astype(np.float16)
        # rotate so the local q-half is cols [0, NLOC); chunk to [128, 8*N]
        xT_rot = np.roll(xT, -h2 * NLOC, axis=1)
        xT_c = np.ascontiguousarray(
            xT_rot.reshape(8, 128, N).transpose(1, 0, 2).reshape(128, 8 * N))
        in_maps.append({"xT": xT_c, "wq": wq, "wk": wk, "wv": wv,
                        "wo": wo, "bo_b": bo_b})

    res = run_bass_kernel_spmd(nc, in_maps, core_ids=list(range(NCORES)),
                               trace=_trace, trace_cores=[0] if _trace else None)
    out = np.empty((B, N, C), dtype=np.float32)
    for c in range(NCORES):
        b, h2 = divmod(c, 2)
        out[b, h2 * NLOC:(h2 + 1) * NLOC, :] = res.results[c]["out"]
    if _trace:
        _cache["last_trace"] = res
    return out
